# revision 1
# baseline (speedup 1.0000x reference)
"""MLA-style attention-score kernel for Trainium2 (8 NeuronCores, SPMD).

Computes, for full inputs
    q_nope_with_k_up [B,H,S,r], compressed_kv [B,S,r], rope_attention [B,H,S,S],
    mask [B,S], have_causal_mask scalar:

    nope   = einsum("bhqr,bkr->bhqk", q, kv)
    scores = (rope + nope) / sqrt(192)   (+ causal / padding masks)
    attn   = softmax(scores, -1)
    out    = einsum("bhqk,bkr->bhqr", attn, kv)        [B,H,S,r] fp32

Sharding: B*H = 64 head-slots, 8 per core; core c owns batch c//2, heads
(c%2)*8..+8, with that batch's compressed_kv replicated on the core.
"""

import math
import os
import sys

import numpy as np

for _p in ("/opt/trn_rl_repo", "/root/.axon_site/_ro/trn_rl_repo"):
    if os.path.isdir(_p) and _p not in sys.path:
        sys.path.insert(0, _p)

import concourse.bass as bass
import concourse.mybir as mybir
import concourse.tile as tile
from concourse import bass_utils
from concourse.vector_clock import ScopedClock

B, H, S, R = 4, 16, 1024, 512
N_CORES = 8
HPC = (B * H) // N_CORES          # heads per core
P = 128                           # partition block
NSB = S // P                      # 8 s-blocks
NRB = R // P                      # 4 r-chunks
SCALE = 1.0 / math.sqrt(64 + 128)
NEG = -1e30
F32 = mybir.dt.float32
F32R = mybir.dt.float32r
BF16 = mybir.dt.bfloat16
AF = mybir.ActivationFunctionType

COMPUTE = os.environ.get("ATTN_COMPUTE", "bf16")  # "f32r" | "bf16"
ROPE_BF16 = os.environ.get("ATTN_ROPE_BF16", "1") == "1"
AV_BF16 = os.environ.get("ATTN_AV_BF16", "1") == "1"
N_WARMUP = int(os.environ.get("ATTN_WARMUP", "26"))


class _TC(tile.TileContext):
    """TileContext whose end-of-kernel drain splits its semaphore waits
    across preceding NOPs (walrus in this image rejects >2 sync waits on
    one Drain)."""

    MAX_WAITS = 1

    def _drain_and_barrier(self, tick_clock, wait_clock):
        nop = self.nc.sync.nop(nofuse=True)
        wait_clock.add_sem_waits(
            nop.ins, ScopedClock({None: tick_clock.global_clock})
        )
        si = nop.ins.sync_info
        waits = list(si.on_wait) if si is not None else []
        if len(waits) > self.MAX_WAITS:
            nop.ins.sync_info = mybir.SyncInfo(
                on_wait=waits[: self.MAX_WAITS], on_update=[]
            )
            for i in range(self.MAX_WAITS, len(waits), self.MAX_WAITS):
                extra = self.nc.sync.nop(nofuse=True)
                extra.ins.sync_info = mybir.SyncInfo(
                    on_wait=waits[i : i + self.MAX_WAITS], on_update=[]
                )
        self.nc.sync.drain()
        self.nc.all_engine_barrier()
        popped = self.nc._tile_sem_poison_stack.pop()
        assert popped is self._sem_poison
        self.nc.clear_and_free_semaphores(list(self.sems.allocated().values()))
        self.nc.all_engine_barrier()


def _mm_cast(ap):
    """View an fp32 AP as float32r for full-rate PE streaming."""
    return ap.bitcast(F32R) if COMPUTE == "f32r" else ap


def _legalize_sync_waits(nc, max_waits=1):
    """walrus in this image allows only `max_waits` sync waits per
    instruction: move excess waits onto NOPs inserted just before the
    instruction on the same engine queue."""
    nid = 0
    for f in nc.m.functions:
        for blk in f.blocks:
            insts = blk.instructions
            out = []
            changed = False
            for inst in insts:
                si = inst.sync_info
                if si is not None and len(si.on_wait) > max_waits:
                    waits = list(si.on_wait)
                    n_pre = len(waits) - max_waits
                    for i in range(0, n_pre, max_waits):
                        nop = mybir.InstNoOp(
                            name=f"waitsplit_{nid}", ins=[], outs=[],
                            engine=inst.engine, bass_nofuse=True,
                            sync_info=mybir.SyncInfo(
                                on_wait=waits[i : min(i + max_waits, n_pre)],
                                on_update=[],
                            ),
                        )
                        nid += 1
                        out.append(nop)
                    inst.sync_info = mybir.SyncInfo(
                        on_wait=waits[n_pre:], on_update=list(si.on_update)
                    )
                    changed = True
                out.append(inst)
            if changed:
                blk.instructions = out


def build_program(causal: bool, hpc: int = HPC, legalize: bool = True):
    bf = COMPUTE == "bf16"
    cdt = BF16 if bf else F32R        # scores-path compute dtype
    nc = bass.Bass()
    q_d = nc.dram_tensor("q", [hpc, S, R], F32, kind="ExternalInput")
    kv_d = nc.dram_tensor("kv", [S, R], F32, kind="ExternalInput")
    rope_d = nc.dram_tensor("rope", [hpc, S, S], F32, kind="ExternalInput")
    cb_d = (
        nc.dram_tensor("cb", [P, P], F32, kind="ExternalInput") if causal else None
    )
    id_d = nc.dram_tensor("ident", [P, P], F32, kind="ExternalInput")
    out_d = nc.dram_tensor("out", [hpc, S, R], F32, kind="ExternalOutput")

    def load(dst, src, ring=None):
        # fp32/f32r load on one of the two HWDGE rings; bf16 cast via SWDGE
        if dst.dtype == BF16:
            nc.gpsimd.dma_start(dst, src)
        elif dst.dtype == F32R:
            (ring or nc.sync).dma_start(dst, src.bitcast(F32R))
        else:
            (ring or nc.sync).dma_start(dst, src)

    def load_split(dst, src, n_axis_len):
        # split a [P, n, ...] load across the two HWDGE rings (descriptor
        # dispatch on each sequencer is ~7us for 2MB; halve it)
        h = n_axis_len // 2
        load(dst[:, :h], src[:, :h], ring=nc.sync)
        load(dst[:, h:], src[:, h:], ring=nc.scalar)

    def chunk_widths(kw):
        # split kw into <=512 chunks, none below 256 (f32r slow under 256)
        out = []
        rem = kw
        while rem > 512:
            take = 512 if rem - 512 >= 256 or rem - 512 == 0 else rem - 256
            out.append(take)
            rem -= take
        out.append(rem)
        return out

    with _TC(nc) as tc:
        with (
            tc.tile_pool(name="const", bufs=1) as const_pool,
            tc.tile_pool(name="kvn", bufs=1) as kvn_pool,
            tc.tile_pool(name="kvt", bufs=1) as kvt_pool,
            tc.tile_pool(name="qn", bufs=2) as qn_pool,
            tc.tile_pool(name="qt", bufs=2) as qt_pool,
            tc.tile_pool(name="rope", bufs=4) as rope_pool,
            tc.tile_pool(name="attn", bufs=4) as attn_pool,
            tc.tile_pool(name="attnT", bufs=4) as attnT_pool,
            tc.tile_pool(name="sums", bufs=6) as sums_pool,
            tc.tile_pool(name="outh", bufs=3) as out_pool,
            tc.tile_pool(name="psc", bufs=3, space="PSUM") as sc_pool,
            tc.tile_pool(name="ptr", bufs=3, space="PSUM") as tr_pool,
            tc.tile_pool(name="pav", bufs=2, space="PSUM") as av_pool,
        ):
            # ---- prologue loads: ident first (first PE op needs it),
            # then kv/q split across both HWDGE rings
            identw = const_pool.tile([P, P], F32R, tag="identw", name="ident_w")
            nc.sync.dma_start(identw[:], id_d[:, :].bitcast(F32R))
            ident = const_pool.tile([P, P], cdt, tag="ident")
            load(ident[:], id_d[:, :])
            ident_b = const_pool.tile([P, P], BF16, tag="identb", name="ident_b")
            nc.gpsimd.dma_start(ident_b[:], id_d[:, :])
            # HAM warm-up: ~5us of back-to-back PE work gated only on the
            # 64KB ident DMA, so the clock gate opens (K=8/8, 2.4GHz)
            # before the first real transpose instead of ~27us in.
            wu = sc_pool.tile([P, P], F32, tag="sc", name="warmup_ps")
            for _ in range(N_WARMUP):
                nc.tensor.matmul(
                    wu[:], identw[:], identw[:], start=True, stop=True,
                )
            if causal:
                cb = const_pool.tile([P, P], BF16, tag="cb")
                nc.gpsimd.dma_start(cb[:], cb_d[:, :])
            # q/kv always ride the fast HWDGE rings as f32r; the bf16 cast
            # happens for free in the transpose's PSUM->SBUF evacuation.
            kv_src = kv_d.rearrange("(n p) r -> p n r", p=P)
            kvA = kvn_pool.tile([P, NSB // 2, R], cdt, tag="kvA", name="kvA")
            load(kvA[:], kv_src[:, : NSB // 2], ring=nc.sync)
            kvB = kvn_pool.tile([P, NSB // 2, R], cdt, tag="kvB", name="kvB")
            load(kvB[:], kv_src[:, NSB // 2 :], ring=nc.scalar)
            kv_half = lambda kb: (kvA if kb < NSB // 2 else kvB)[:, kb % (NSB // 2), :]

            ncopy = [0]

            def pcopy(dst, src):
                # PSUM -> SBUF evacuation, alternating DVE / ACT
                if ncopy[0] % 2 == 0:
                    nc.vector.tensor_copy(dst, src)
                else:
                    nc.scalar.copy(dst, src)
                ncopy[0] += 1

            def transpose_into(dst_tile, dst_off, blocks, idt):
                """PE-transpose [P,P] `blocks` (list of APs), writing the
                j-th transposed block at dst_tile[:, dst_off + j*P]."""
                psdt = blocks[0].dtype
                ps = tr_pool.tile([P, 512], psdt, tag="tr", name="tr_ps")
                for j, srcb in enumerate(blocks):
                    nc.tensor.matmul(
                        ps[:, j * P : (j + 1) * P], srcb, idt[:],
                        is_transpose=True,
                    )
                w = len(blocks) * P
                src_ap = ps[:, :w]
                if psdt == F32R and dst_tile.dtype != F32R:
                    src_ap = src_ap.bitcast(F32)   # evacuation cast f32->bf16
                pcopy(dst_tile[:, dst_off : dst_off + w], src_ap)

            kv_t = kvt_pool.tile([P, NRB, S], cdt)
            for g in range(NSB // 4):
                kvh = kvA if g == 0 else kvB
                for rb in range(NRB):
                    transpose_into(
                        kv_t[:, rb], g * 4 * P,
                        [kvh[:, j, rb * P : (rb + 1) * P] for j in range(4)],
                        ident,
                    )

            # ---- software-pipelined head/qb loop (stage 2 lags one step)
            steps = [(h, qb) for h in range(hpc) for qb in range(NSB)]
            carry = {}   # step index -> dict of live tiles
            heads = {}   # h -> dict(qT=..., q_nat=...)
            ropes = {}   # step index -> rope tile

            def q_load(h, part=None):
                # split halves so the 2MB cast doesn't block rope chunks
                # queued behind it on the same SWDGE FIFO
                q_src = q_d[h].rearrange("(n p) r -> p n r", p=P)
                if part in (None, 0):
                    qA = qn_pool.tile([P, NSB // 2, R], cdt, tag="qA", name="qA")
                    load(qA[:], q_src[:, : NSB // 2], ring=nc.sync)
                    heads.setdefault(h, {})["qA"] = qA
                if part in (None, 1):
                    qB = qn_pool.tile([P, NSB // 2, R], cdt, tag="qB", name="qB")
                    load(qB[:], q_src[:, NSB // 2 :], ring=nc.scalar)
                    heads.setdefault(h, {})["qB"] = qB

            def rope_load(i):
                h, qb = steps[i]
                nk = (qb + 1) if causal else NSB
                rope_t = rope_pool.tile([P, S], BF16 if ROPE_BF16 else cdt,
                                        tag="rope", name="rope_t")
                load(rope_t[:, : nk * P],
                     rope_d[h, qb * P : (qb + 1) * P, 0 : nk * P])
                ropes[i] = rope_t

            def stage1(i):
                h, qb = steps[i]
                if qb == 0:
                    qT = qt_pool.tile([P, NRB, S], cdt, name="qT")
                    for g in range(NSB // 4):
                        qh = heads[h]["qA"] if g == 0 else heads[h]["qB"]
                        for rb in range(NRB):
                            transpose_into(
                                qT[:, rb], g * 4 * P,
                                [qh[:, j, rb * P : (rb + 1) * P] for j in range(4)],
                                ident,
                            )
                    heads[h]["qT"] = qT
                qT = heads[h]["qT"]
                nk = (qb + 1) if causal else NSB
                kw = nk * P
                rope_t = ropes.pop(i)
                attn = attn_pool.tile([P, S], BF16 if AV_BF16 else cdt,
                                      tag="attn", name="attn_t")
                sums = []
                c0 = 0
                for w in chunk_widths(kw):
                    ps = sc_pool.tile([P, 512], F32, tag="sc", name="sc_ps")
                    # rope -> psum via bf16 identity matmul (starts the group)
                    nc.tensor.matmul(
                        ps[:, :w],
                        ident_b[:] if ROPE_BF16 else ident[:],
                        rope_t[:, c0 : c0 + w],
                        start=True, stop=False,
                    )
                    if causal and c0 <= (nk - 1) * P < c0 + w:
                        d0 = (nk - 1) * P - c0
                        nc.tensor.matmul(
                            ps[:, d0 : d0 + P], ident_b[:], cb[:],
                            start=False, stop=False,
                        )
                    for rb in range(NRB):
                        nc.tensor.matmul(
                            ps[:, :w],
                            qT[:, rb, qb * P : (qb + 1) * P],
                            kv_t[:, rb, c0 : c0 + w],
                            start=False, stop=(rb == NRB - 1),
                        )
                    s_t = sums_pool.tile([P, 1], F32, tag="sums", name="sums_t")
                    nc.scalar.activation(
                        attn[:, c0 : c0 + w], ps[:, :w], AF.Exp,
                        scale=SCALE, accum_out=s_t[:],
                    )
                    sums.append(s_t)
                    c0 += w
                if len(sums) > 1:
                    tot = sums_pool.tile([P, 1], F32, tag="sums", name="tot_t")
                    nc.vector.tensor_add(tot[:], sums[0][:], sums[1][:])
                    sums = [tot]
                carry[i] = {"attn": attn, "sum": sums[0], "nk": nk}

            def stage2(i):
                h, qb = steps[i]
                st = carry.pop(i)
                attn, nk = st["attn"], st["nk"]
                groups = []
                for g in range(0, nk, 4):
                    jcnt = min(4, nk - g)
                    at_g = attnT_pool.tile([P, 512], BF16 if AV_BF16 else cdt,
                                           tag="attnT", name="attnT_t")
                    transpose_into(
                        at_g, 0,
                        [attn[:, (g + j) * P : (g + j + 1) * P] for j in range(jcnt)],
                        ident_b if (AV_BF16 or bf) else ident,
                    )
                    groups.append(at_g)
                av = av_pool.tile([P, R], F32, tag="av", name="av_ps")
                for kb in range(nk):
                    nc.tensor.matmul(
                        av[:],
                        groups[kb // 4][:, (kb % 4) * P : (kb % 4 + 1) * P],
                        kv_bf[:, kb, :] if (AV_BF16 and not bf) else kv_half(kb),
                        start=(kb == 0), stop=(kb == nk - 1),
                    )
                recip = sums_pool.tile([P, 1], F32, tag="recip", name="recip_t")
                nc.vector.reciprocal(recip[:], st["sum"][:])
                out_t = out_pool.tile([P, R], F32, tag="outh", name="out_t")
                if i % 2 == 0:
                    nc.vector.tensor_scalar_mul(out_t[:], av[:], recip[:])
                else:
                    nc.scalar.activation(out_t[:], av[:], AF.Copy, scale=recip[:])
                nc.sync.dma_start(
                    out_d[h].rearrange("(n p) r -> p n r", p=P)[:, qb], out_t[:]
                )
                if qb == NSB - 1:
                    heads.pop(h)

            SKEW = 2
            q_load(0)
            if AV_BF16 and not bf:
                kv_bf = kvn_pool.tile([P, NSB, R], BF16, name="kv_bf")
                nc.gpsimd.dma_start(kv_bf[:], kv_src)
            for j in range(min(2, len(steps))):
                rope_load(j)
            for i in range(len(steps) + SKEW):
                if i < len(steps):
                    h, qb = steps[i]
                    if qb == 4 and h + 1 < hpc:
                        q_load(h + 1)
                    if i + 2 < len(steps):
                        rope_load(i + 2)
                    stage1(i)
                if i >= SKEW:
                    stage2(i - SKEW)

    if legalize:
        _legalize_sync_waits(nc)
    return nc


_CACHE = {}


def _program(causal: bool):
    key = (causal, COMPUTE)
    if key not in _CACHE:
        _CACHE[key] = build_program(causal)
    return _CACHE[key]


def kernel(q_nope_with_k_up, compressed_kv, rope_attention, mask,
           have_causal_mask) -> np.ndarray:
    q = np.asarray(q_nope_with_k_up, dtype=np.float32)
    kv = np.asarray(compressed_kv, dtype=np.float32)
    rope = np.asarray(rope_attention, dtype=np.float32)
    causal = bool(int(np.asarray(have_causal_mask)))

    if mask is not None:
        m = np.asarray(mask)
        if m.any():
            # padding mask (all-zero in practice): fold into a rope copy
            rope = rope + np.where(m, NEG, 0.0).astype(np.float32)[:, None, None, :]

    cb = np.where(
        np.triu(np.ones((P, P), np.bool_), 1), np.float32(NEG), np.float32(0)
    ).astype(np.float32)
    ident_np = np.eye(P, dtype=np.float32)

    nc = _program(causal)
    in_maps = []
    for c in range(N_CORES):
        b, h0 = c // (H // HPC), (c % (H // HPC)) * HPC
        im = {
            "q": q[b, h0 : h0 + HPC],
            "kv": kv[b],
            "rope": rope[b, h0 : h0 + HPC],
            "ident": ident_np,
        }
        if causal:
            im["cb"] = cb
        in_maps.append(im)

    res = bass_utils.run_bass_kernel_spmd(nc, in_maps, core_ids=list(range(N_CORES)))

    out = np.empty((B, H, S, R), np.float32)
    for c in range(N_CORES):
        b, h0 = c // (H // HPC), (c % (H // HPC)) * HPC
        out[b, h0 : h0 + HPC] = res.results[c]["out"]
    return out

